# revision 2
# baseline (speedup 1.0000x reference)
"""Trainium2 Bass kernel for nn_DecoderRNN (240-step LSTM decoder, B=512, H=2048).

Sharding: 8-way tensor parallel. Each core owns 1024 of the 8192 gate rows
(256 rows of each of i/f/g/o) and the matching 256 rows of h/c/z. All weights
stay SBUF-resident in bf16. Per step: one AllGather of h (256KB/rank) and one
of z; fc2 is computed replicated on every core so no AllReduce is needed.
The one-hot class encoding is folded into the gates matmul as a K=40 tile and
all biases fold into scalar-engine activation instructions.
"""

import sys

if "/opt/trn_rl_repo" not in sys.path:
    sys.path.insert(0, "/opt/trn_rl_repo")

import numpy as np
import ml_dtypes

B = 512
OUT = 165
H = 2048
G4 = 4 * H
NCLS = 40
NC = 8
BL = B // NC  # batch columns stored per core
KT = H // 128  # 16 k-tiles over the hidden dim
MR = [128, OUT - 128]  # row-tile sizes for the 165-row out/fc2 dim

_CACHE = {}


def _build(L):
    import concourse.bacc as bacc
    import concourse.mybir as mybir
    import concourse.tile as tile
    from concourse.bass import ds
    from contextlib import ExitStack

    f32 = mybir.dt.float32
    bf16 = mybir.dt.bfloat16
    AF = mybir.ActivationFunctionType
    RG = [list(range(NC))]

    nc = bacc.Bacc("TRN2", target_bir_lowering=False, debug=False, num_devices=NC)

    whh_d = nc.dram_tensor("whh", [H, 1024], bf16, kind="ExternalInput")
    wih_d = nc.dram_tensor("wih", [OUT, 1024], bf16, kind="ExternalInput")
    moh_d = nc.dram_tensor("moh", [NCLS, 1024], bf16, kind="ExternalInput")
    wfc1_d = nc.dram_tensor("wfc1", [H, 256], bf16, kind="ExternalInput")
    wfc2_d = nc.dram_tensor("wfc2", [H, OUT], bf16, kind="ExternalInput")
    onehot_d = nc.dram_tensor("onehot", [NCLS, B], bf16, kind="ExternalInput")
    bgates_d = nc.dram_tensor("bgates", [128, 8], f32, kind="ExternalInput")
    bz_d = nc.dram_tensor("bz", [128, 2], f32, kind="ExternalInput")
    bo_d = nc.dram_tensor("bo", [128, 2], f32, kind="ExternalInput")
    h0_d = nc.dram_tensor("h0", [H, B], bf16, kind="ExternalInput")
    c0_d = nc.dram_tensor("c0", [256, B], f32, kind="ExternalInput")
    out0_d = nc.dram_tensor("out0", [OUT, B], bf16, kind="ExternalInput")
    outs_d = nc.dram_tensor("outs", [L, OUT, BL], f32, kind="ExternalOutput")

    with tile.TileContext(nc) as tc, ExitStack() as ctx:
        const = ctx.enter_context(tc.tile_pool(name="const", bufs=1))
        state = ctx.enter_context(tc.tile_pool(name="state", bufs=2))
        work = ctx.enter_context(tc.tile_pool(name="work", bufs=2))
        psum = ctx.enter_context(tc.tile_pool(name="psum", bufs=8, space="PSUM"))
        dram = ctx.enter_context(tc.tile_pool(name="dram", bufs=2, space="DRAM"))

        pid = nc.gpsimd.partition_id()
        col0 = pid * BL

        # ---- constants into SBUF
        whh_sb = const.tile([128, KT * 1024], bf16, name="whh_sb")
        nc.sync.dma_start(
            whh_sb.rearrange("p (k m) -> p k m", k=KT),
            whh_d.ap().rearrange("(k p) m -> p k m", p=128),
        )
        wih0_sb = const.tile([128, 1024], bf16, name="wih0_sb")
        nc.sync.dma_start(wih0_sb[:], wih_d.ap()[0:128, :])
        wih1_sb = const.tile([37, 1024], bf16, name="wih1_sb")
        nc.sync.dma_start(wih1_sb[:], wih_d.ap()[128:165, :])
        moh_sb = const.tile([NCLS, 1024], bf16, name="moh_sb")
        nc.sync.dma_start(moh_sb[:], moh_d.ap()[:, :])
        wfc1_sb = const.tile([128, KT * 256], bf16, name="wfc1_sb")
        nc.sync.dma_start(
            wfc1_sb.rearrange("p (k m) -> p k m", k=KT),
            wfc1_d.ap().rearrange("(k p) m -> p k m", p=128),
        )
        wfc2_sb = const.tile([128, KT * OUT], bf16, name="wfc2_sb")
        nc.sync.dma_start(
            wfc2_sb.rearrange("p (k m) -> p k m", k=KT),
            wfc2_d.ap().rearrange("(k p) m -> p k m", p=128),
        )
        onehot_sb = const.tile([NCLS, B], bf16, name="onehot_sb")
        nc.sync.dma_start(onehot_sb[:], onehot_d.ap()[:, :])
        bg_sb = const.tile([128, 8], f32, name="bg_sb")
        nc.sync.dma_start(bg_sb[:], bgates_d.ap()[:, :])
        bz_sb = const.tile([128, 2], f32, name="bz_sb")
        nc.sync.dma_start(bz_sb[:], bz_d.ap()[:, :])
        bo_sb = const.tile([128, 2], f32, name="bo_sb")
        nc.sync.dma_start(bo_sb[:], bo_d.ap()[:, :])

        def load_cat(dst, src_ap):
            # dst: SBUF [128, KT*B]; src: DRAM [H, B] (KT row-blocks of 128)
            for q in range(4):
                nc.sync.dma_start(
                    dst[:, q * 4 * B:(q + 1) * 4 * B].rearrange("p (k n) -> p k n", k=4),
                    src_ap[q * 512:(q + 1) * 512, :].rearrange("(k p) n -> p k n", p=128),
                )

        # ---- initial state
        hcat = state.tile([128, KT * B], bf16, tag="hcat", name="hcat_init")
        load_cat(hcat, h0_d.ap())
        outb0 = state.tile([128, B], bf16, tag="outb0", name="outb0_init")
        nc.sync.dma_start(outb0[:], out0_d.ap()[0:128, :])
        outb1 = state.tile([37, B], bf16, tag="outb1", name="outb1_init")
        nc.sync.dma_start(outb1[:], out0_d.ap()[128:165, :])
        c_prev = []
        for g in range(2):
            ct = state.tile([128, B], f32, tag=f"c{g}", name=f"c{g}_init")
            nc.sync.dma_start(ct[:], c0_d.ap()[g * 128:(g + 1) * 128, :])
            c_prev.append(ct)

        for t in range(L):
            # ---- gates matmuls: 8 m-tiles x (16 Whh + 2 Wih + 1 onehot) k-tiles
            psg = []
            for mt in range(8):
                ps = psum.tile([128, B], f32, tag="ps", name=f"psg_{t}_{mt}")
                psg.append(ps)
                for ki in range(KT):
                    nc.tensor.matmul(
                        ps[:],
                        whh_sb[:, ki * 1024 + mt * 128: ki * 1024 + (mt + 1) * 128],
                        hcat[:, ki * B:(ki + 1) * B],
                        start=(ki == 0),
                        stop=False,
                    )
            for mt in range(8):
                ps = psg[mt]
                nc.tensor.matmul(ps[:], wih0_sb[:, mt * 128:(mt + 1) * 128], outb0[:],
                                 start=False, stop=False)
                nc.tensor.matmul(ps[:], wih1_sb[:, mt * 128:(mt + 1) * 128], outb1[:],
                                 start=False, stop=False)
                nc.tensor.matmul(ps[:], moh_sb[:, mt * 128:(mt + 1) * 128], onehot_sb[:],
                                 start=False, stop=True)

            # ---- LSTM elementwise per 128-row group; h -> AG bounce
            hb_in = dram.tile([256, B], bf16, tag="hbin", name=f"hbin_{t}")
            c_new_list = []
            for g in range(2):
                pi, pf, pg_, po = psg[g * 4: g * 4 + 4]
                si = work.tile([128, B], f32, tag="si", name=f"si_{t}_{g}")
                nc.scalar.activation(si[:], pi[:], AF.Sigmoid, bias=bg_sb[:, 4 * g + 0: 4 * g + 1])
                sf = work.tile([128, B], f32, tag="sf", name=f"sf_{t}_{g}")
                nc.scalar.activation(sf[:], pf[:], AF.Sigmoid, bias=bg_sb[:, 4 * g + 1: 4 * g + 2])
                tg = work.tile([128, B], f32, tag="tg", name=f"tg_{t}_{g}")
                nc.scalar.activation(tg[:], pg_[:], AF.Tanh, bias=bg_sb[:, 4 * g + 2: 4 * g + 3])
                so = work.tile([128, B], f32, tag="so", name=f"so_{t}_{g}")
                nc.scalar.activation(so[:], po[:], AF.Sigmoid, bias=bg_sb[:, 4 * g + 3: 4 * g + 4])
                m1 = work.tile([128, B], f32, tag="m1", name=f"m1_{t}_{g}")
                nc.vector.tensor_mul(m1[:], si[:], tg[:])
                m2 = work.tile([128, B], f32, tag="m2", name=f"m2_{t}_{g}")
                nc.vector.tensor_mul(m2[:], sf[:], c_prev[g][:])
                c_new = state.tile([128, B], f32, tag=f"c{g}", name=f"c{g}_{t}")
                nc.vector.tensor_add(c_new[:], m1[:], m2[:])
                th = work.tile([128, B], f32, tag="th", name=f"th_{t}_{g}")
                nc.scalar.activation(th[:], c_new[:], AF.Tanh)
                hn = work.tile([128, B], bf16, tag="hn", name=f"hn_{t}_{g}")
                nc.vector.tensor_mul(hn[:], so[:], th[:])
                nc.sync.dma_start(hb_in[g * 128:(g + 1) * 128, :], hn[:])
                c_new_list.append(c_new)
            c_prev = c_new_list

            # ---- AllGather h, reload as [128, KT*B]
            hb_out = dram.tile([H, B], bf16, tag="hbout", name=f"hbout_{t}",
                               addr_space="Shared")
            nc.gpsimd.collective_compute(
                "AllGather", mybir.AluOpType.bypass, replica_groups=RG,
                ins=[hb_in.opt()], outs=[hb_out.opt()],
            )
            hcat_new = state.tile([128, KT * B], bf16, tag="hcat", name=f"hcat_{t}")
            load_cat(hcat_new, hb_out)

            # ---- fc1 (own 256 rows) + relu -> z AG bounce
            zb_in = dram.tile([256, B], bf16, tag="zbin", name=f"zbin_{t}")
            for mt in range(2):
                ps = psum.tile([128, B], f32, tag="ps", name=f"psz_{t}_{mt}")
                for ki in range(KT):
                    nc.tensor.matmul(
                        ps[:],
                        wfc1_sb[:, ki * 256 + mt * 128: ki * 256 + (mt + 1) * 128],
                        hcat_new[:, ki * B:(ki + 1) * B],
                        start=(ki == 0),
                        stop=(ki == KT - 1),
                    )
                zb = work.tile([128, B], bf16, tag=f"zb{mt}", name=f"zb_{t}_{mt}")
                nc.scalar.activation(zb[:], ps[:], AF.Relu, bias=bz_sb[:, mt:mt + 1])
                nc.sync.dma_start(zb_in[mt * 128:(mt + 1) * 128, :], zb[:])

            zb_out = dram.tile([H, B], bf16, tag="zbout", name=f"zbout_{t}",
                               addr_space="Shared")
            nc.gpsimd.collective_compute(
                "AllGather", mybir.AluOpType.bypass, replica_groups=RG,
                ins=[zb_in.opt()], outs=[zb_out.opt()],
            )
            zcat = state.tile([128, KT * B], bf16, tag="zcat", name=f"zcat_{t}")
            load_cat(zcat, zb_out)

            # ---- fc2 (full 165 rows, replicated) + out store + bf16 copy
            new_outb = []
            for mt in range(2):
                mr = MR[mt]
                ps = psum.tile([128, B], f32, tag="ps", name=f"pso_{t}_{mt}")
                for ki in range(KT):
                    nc.tensor.matmul(
                        ps[:mr],
                        wfc2_sb[:, ki * OUT + mt * 128: ki * OUT + mt * 128 + mr],
                        zcat[:, ki * B:(ki + 1) * B],
                        start=(ki == 0),
                        stop=(ki == KT - 1),
                    )
                of = work.tile([128, B], f32, tag=f"of{mt}", name=f"of_{t}_{mt}")
                nc.scalar.activation(of[:mr], ps[:mr], AF.Identity, bias=bo_sb[:mr, mt:mt + 1])
                nc.gpsimd.dma_start(
                    outs_d.ap()[t, mt * 128: mt * 128 + mr, :],
                    of[:mr, ds(col0, BL)],
                )
                ob = state.tile([mr, B], bf16, tag=f"outb{mt}", name=f"outb{mt}_{t}")
                nc.vector.tensor_copy(ob[:], of[:mr])
                new_outb.append(ob)
            outb0, outb1 = new_outb
            hcat = hcat_new

    nc.compile()
    return nc


def _prepare_in_maps(inputs):
    bf = ml_dtypes.bfloat16
    f = {k: np.asarray(v) for k, v in inputs.items()}
    W_enc = f["W_enc"].astype(np.float32)
    b_enc = f["b_enc"].astype(np.float32)
    W_ih = f["W_ih"].astype(np.float32)
    b_ih = f["b_ih"].astype(np.float32)
    W_hh = f["W_hh"].astype(np.float32)
    b_hh = f["b_hh"].astype(np.float32)
    W_fc1 = f["W_fc1"].astype(np.float32)
    b_fc1 = f["b_fc1"].astype(np.float32)
    W_fc2 = f["W_fc2"].astype(np.float32)
    b_fc2 = f["b_fc2"].astype(np.float32)
    W_inh = f["W_inh"].astype(np.float32)
    b_inh = f["b_inh"].astype(np.float32)
    W_inc = f["W_inc"].astype(np.float32)
    b_inc = f["b_inc"].astype(np.float32)
    labels = f["labels"].astype(np.int64)
    x = f["inputs"].astype(np.float32)

    frame0 = x.reshape(B, OUT)
    h0 = frame0 @ W_inh.T + b_inh            # [B, H]
    c0 = frame0 @ W_inc.T + b_inc            # [B, H]
    onehot = np.zeros((NCLS, B), np.float32)
    onehot[labels, np.arange(B)] = 1.0
    M1 = W_ih[:, OUT:] @ W_enc               # [4H, NCLS]
    bias_gates = b_ih + b_hh + W_ih[:, OUT:] @ b_enc  # [4H]

    in_maps = []
    for j in range(NC):
        mt = np.arange(8)
        gt, g = mt % 4, mt // 4
        rows = (gt[:, None] * H + j * 256 + g[:, None] * 128 + np.arange(128)[None, :]).reshape(-1)
        zrows = j * 256 + np.arange(256)
        bg = bias_gates[rows].reshape(8, 128).T.copy()          # [128, 8]
        bzv = b_fc1[zrows].reshape(2, 128).T.copy()             # [128, 2]
        bov = np.zeros((128, 2), np.float32)
        bov[:, 0] = b_fc2[:128]
        bov[:MR[1], 1] = b_fc2[128:]
        in_maps.append({
            "whh": np.ascontiguousarray(W_hh[rows].T).astype(bf),
            "wih": np.ascontiguousarray(W_ih[rows, :OUT].T).astype(bf),
            "moh": np.ascontiguousarray(M1[rows].T).astype(bf),
            "wfc1": np.ascontiguousarray(W_fc1[zrows].T).astype(bf),
            "wfc2": np.ascontiguousarray(W_fc2.T).astype(bf),
            "onehot": onehot.astype(bf),
            "bgates": bg,
            "bz": bzv,
            "bo": bov,
            "h0": np.ascontiguousarray(h0.T).astype(bf),
            "c0": np.ascontiguousarray(c0.T[zrows]).astype(np.float32),
            "out0": np.ascontiguousarray(frame0.T).astype(bf),
        })
    return in_maps


def _get_program(L):
    if L not in _CACHE:
        _CACHE[L] = _build(L)
    return _CACHE[L]


def kernel(**inputs):
    from concourse.bass_utils import run_bass_kernel_spmd

    L = int(np.asarray(inputs["length"]))
    x = np.asarray(inputs["inputs"])
    Bq, J, D = x.shape
    assert (Bq, J * D) == (B, OUT)

    nc = _get_program(L)
    in_maps = _prepare_in_maps(inputs)
    res = run_bass_kernel_spmd(nc, in_maps, core_ids=list(range(NC)))
    # core j returns [L, OUT, BL] covering batch columns j*BL:(j+1)*BL
    full = np.concatenate([res.results[j]["outs"] for j in range(NC)], axis=2)
    out = np.transpose(full, (2, 0, 1)).reshape(B, L, J, D).astype(np.float32)
    return out


# revision 6
# speedup vs baseline: 1.3248x; 1.3248x over previous
"""Trainium2 Bass kernel for nn_DecoderRNN (240-step LSTM decoder, B=512, H=2048).

Sharding: 8-way tensor parallel. Each core owns 1024 of the 8192 gate rows
(256 rows of each of i/f/g/o) and the matching 256 rows of h/c/z. All weights
stay SBUF-resident in bf16. Per step: one AllGather of h (256KB/rank) and one
of z; fc2 is computed replicated on every core so no AllReduce is needed.
The one-hot class encoding is folded into the gates matmul as a K=40 tile and
all biases fold into scalar-engine activation instructions.
"""

import sys

if "/opt/trn_rl_repo" not in sys.path:
    sys.path.insert(0, "/opt/trn_rl_repo")

import numpy as np
import ml_dtypes

B = 512
OUT = 165
H = 2048
G4 = 4 * H
NCLS = 40
NC = 8
BL = B // NC  # batch columns stored per core
KT = H // 128  # 16 k-tiles over the hidden dim
MR = [128, OUT - 128]  # row-tile sizes for the 165-row out/fc2 dim

_CACHE = {}

# Ablation knobs (timing experiments only; ABLATE_CC breaks correctness).
ABLATE_CC = False      # replace AllGathers with local own-slice DMA copy
ABLATE_STORE = False   # skip the dynamic-offset out stores


def _build(L):
    import concourse.bacc as bacc
    import concourse.mybir as mybir
    import concourse.tile as tile
    from concourse.bass import ds
    from contextlib import ExitStack

    f32 = mybir.dt.float32
    bf16 = mybir.dt.bfloat16
    AF = mybir.ActivationFunctionType
    RG = [list(range(NC))]

    nc = bacc.Bacc("TRN2", target_bir_lowering=False, debug=False, num_devices=NC)

    whh_d = nc.dram_tensor("whh", [H, 1024], bf16, kind="ExternalInput")
    wih_d = nc.dram_tensor("wih", [OUT, 1024], bf16, kind="ExternalInput")
    moh_d = nc.dram_tensor("moh", [NCLS, 1024], bf16, kind="ExternalInput")
    wfc1_d = nc.dram_tensor("wfc1", [H, 256], bf16, kind="ExternalInput")
    wfc2_d = nc.dram_tensor("wfc2", [H, OUT], bf16, kind="ExternalInput")
    onehot_d = nc.dram_tensor("onehot", [NCLS, B], bf16, kind="ExternalInput")
    bgates_d = nc.dram_tensor("bgates", [128, 8], f32, kind="ExternalInput")
    bz_d = nc.dram_tensor("bz", [128, 2], f32, kind="ExternalInput")
    bo_d = nc.dram_tensor("bo", [128, 2], f32, kind="ExternalInput")
    h0_d = nc.dram_tensor("h0", [H, B], bf16, kind="ExternalInput")
    c0_d = nc.dram_tensor("c0", [256, B], f32, kind="ExternalInput")
    out0_d = nc.dram_tensor("out0", [OUT, B], bf16, kind="ExternalInput")
    outs_d = nc.dram_tensor("outs", [L, OUT, BL], f32, kind="ExternalOutput")

    with tile.TileContext(nc) as tc, ExitStack() as ctx:
        const = ctx.enter_context(tc.tile_pool(name="const", bufs=1))
        state = ctx.enter_context(tc.tile_pool(name="state", bufs=2))
        work = ctx.enter_context(tc.tile_pool(name="work", bufs=2))
        psum = ctx.enter_context(tc.tile_pool(name="psum", bufs=8, space="PSUM"))
        dram = ctx.enter_context(tc.tile_pool(name="dram", bufs=2, space="DRAM"))

        pid = nc.gpsimd.partition_id()
        col0 = pid * BL

        # ---- constants into SBUF
        whh_sb = const.tile([128, KT * 1024], bf16, name="whh_sb")
        nc.sync.dma_start(
            whh_sb.rearrange("p (k m) -> p k m", k=KT),
            whh_d.ap().rearrange("(k p) m -> p k m", p=128),
        )
        wih0_sb = const.tile([128, 1024], bf16, name="wih0_sb")
        nc.sync.dma_start(wih0_sb[:], wih_d.ap()[0:128, :])
        wih1_sb = const.tile([37, 1024], bf16, name="wih1_sb")
        nc.sync.dma_start(wih1_sb[:], wih_d.ap()[128:165, :])
        moh_sb = const.tile([NCLS, 1024], bf16, name="moh_sb")
        nc.sync.dma_start(moh_sb[:], moh_d.ap()[:, :])
        wfc1_sb = const.tile([128, KT * 256], bf16, name="wfc1_sb")
        nc.sync.dma_start(
            wfc1_sb.rearrange("p (k m) -> p k m", k=KT),
            wfc1_d.ap().rearrange("(k p) m -> p k m", p=128),
        )
        wfc2_sb = const.tile([128, KT * OUT], bf16, name="wfc2_sb")
        nc.sync.dma_start(
            wfc2_sb.rearrange("p (k m) -> p k m", k=KT),
            wfc2_d.ap().rearrange("(k p) m -> p k m", p=128),
        )
        onehot_sb = const.tile([NCLS, B], bf16, name="onehot_sb")
        nc.sync.dma_start(onehot_sb[:], onehot_d.ap()[:, :])
        bg_sb = const.tile([128, 8], f32, name="bg_sb")
        nc.sync.dma_start(bg_sb[:], bgates_d.ap()[:, :])
        bz_sb = const.tile([128, 2], f32, name="bz_sb")
        nc.sync.dma_start(bz_sb[:], bz_d.ap()[:, :])
        bo_sb = const.tile([128, 2], f32, name="bo_sb")
        nc.sync.dma_start(bo_sb[:], bo_d.ap()[:, :])

        def load_cat(dst, src_ap):
            # dst: SBUF [128, KT*B]; src: DRAM [H, B] (KT row-blocks of 128)
            for q in range(4):
                nc.sync.dma_start(
                    dst[:, q * 4 * B:(q + 1) * 4 * B].rearrange("p (k n) -> p k n", k=4),
                    src_ap[q * 512:(q + 1) * 512, :].rearrange("(k p) n -> p k n", p=128),
                )

        # ---- initial state
        hcat = state.tile([128, KT * B], bf16, tag="hcat", name="hcat_init")
        load_cat(hcat, h0_d.ap())
        outb0 = state.tile([128, B], bf16, tag="outb0", name="outb0_init")
        nc.sync.dma_start(outb0[:], out0_d.ap()[0:128, :])
        outb1 = state.tile([37, B], bf16, tag="outb1", name="outb1_init")
        nc.sync.dma_start(outb1[:], out0_d.ap()[128:165, :])
        c_prev = []
        for g in range(2):
            ct = state.tile([128, B], f32, tag=f"c{g}", name=f"c{g}_init")
            nc.sync.dma_start(ct[:], c0_d.ap()[g * 128:(g + 1) * 128, :])
            c_prev.append(ct)

        for t in range(L):
            # ---- gates matmuls: 8 m-tiles x (16 Whh + 2 Wih + 1 onehot) k-tiles
            psg = []
            for mt in range(8):
                ps = psum.tile([128, B], f32, tag="ps", name=f"psg_{t}_{mt}")
                psg.append(ps)
                for ki in range(KT):
                    nc.tensor.matmul(
                        ps[:],
                        whh_sb[:, ki * 1024 + mt * 128: ki * 1024 + (mt + 1) * 128],
                        hcat[:, ki * B:(ki + 1) * B],
                        start=(ki == 0),
                        stop=False,
                    )
            for mt in range(8):
                ps = psg[mt]
                nc.tensor.matmul(ps[:], wih0_sb[:, mt * 128:(mt + 1) * 128], outb0[:],
                                 start=False, stop=False)
                nc.tensor.matmul(ps[:], wih1_sb[:, mt * 128:(mt + 1) * 128], outb1[:],
                                 start=False, stop=False)
                nc.tensor.matmul(ps[:], moh_sb[:, mt * 128:(mt + 1) * 128], onehot_sb[:],
                                 start=False, stop=True)

            # ---- LSTM elementwise per 128-row group; h -> AG bounce
            hb_in = dram.tile([256, B], bf16, tag="hbin", name=f"hbin_{t}")
            c_new_list = []
            for g in range(2):
                pi, pf, pg_, po = psg[g * 4: g * 4 + 4]
                si = work.tile([128, B], f32, tag="si", name=f"si_{t}_{g}")
                nc.scalar.activation(si[:], pi[:], AF.Sigmoid, bias=bg_sb[:, 4 * g + 0: 4 * g + 1])
                sf = work.tile([128, B], f32, tag="sf", name=f"sf_{t}_{g}")
                nc.scalar.activation(sf[:], pf[:], AF.Sigmoid, bias=bg_sb[:, 4 * g + 1: 4 * g + 2])
                tg = work.tile([128, B], f32, tag="tg", name=f"tg_{t}_{g}")
                nc.scalar.activation(tg[:], pg_[:], AF.Tanh, bias=bg_sb[:, 4 * g + 2: 4 * g + 3])
                so = work.tile([128, B], f32, tag="so", name=f"so_{t}_{g}")
                nc.scalar.activation(so[:], po[:], AF.Sigmoid, bias=bg_sb[:, 4 * g + 3: 4 * g + 4])
                m1 = work.tile([128, B], f32, tag="m1", name=f"m1_{t}_{g}")
                nc.vector.tensor_mul(m1[:], si[:], tg[:])
                m2 = work.tile([128, B], f32, tag="m2", name=f"m2_{t}_{g}")
                nc.vector.tensor_mul(m2[:], sf[:], c_prev[g][:])
                c_new = state.tile([128, B], f32, tag=f"c{g}", name=f"c{g}_{t}")
                nc.vector.tensor_add(c_new[:], m1[:], m2[:])
                th = work.tile([128, B], f32, tag="th", name=f"th_{t}_{g}")
                nc.scalar.activation(th[:], c_new[:], AF.Tanh)
                hn = work.tile([128, B], bf16, tag="hn", name=f"hn_{t}_{g}")
                nc.vector.tensor_mul(hn[:], so[:], th[:])
                nc.sync.dma_start(hb_in[g * 128:(g + 1) * 128, :], hn[:])
                c_new_list.append(c_new)
            c_prev = c_new_list

            # ---- AllGather h, reload as [128, KT*B]
            hb_out = dram.tile([H, B], bf16, tag="hbout", name=f"hbout_{t}",
                               addr_space="Shared")
            if ABLATE_CC:
                nc.sync.dma_start(hb_out[0:256, :], hb_in[:])
            else:
                nc.gpsimd.collective_compute(
                    "AllGather", mybir.AluOpType.bypass, replica_groups=RG,
                    ins=[hb_in.opt()], outs=[hb_out.opt()],
                )
            hcat_new = state.tile([128, KT * B], bf16, tag="hcat", name=f"hcat_{t}")
            load_cat(hcat_new, hb_out)

            # ---- fc1 (own 256 rows) + relu -> z AG bounce
            zb_in = dram.tile([256, B], bf16, tag="zbin", name=f"zbin_{t}")
            for mt in range(2):
                ps = psum.tile([128, B], f32, tag="ps", name=f"psz_{t}_{mt}")
                for ki in range(KT):
                    nc.tensor.matmul(
                        ps[:],
                        wfc1_sb[:, ki * 256 + mt * 128: ki * 256 + (mt + 1) * 128],
                        hcat_new[:, ki * B:(ki + 1) * B],
                        start=(ki == 0),
                        stop=(ki == KT - 1),
                    )
                zb = work.tile([128, B], bf16, tag=f"zb{mt}", name=f"zb_{t}_{mt}")
                nc.scalar.activation(zb[:], ps[:], AF.Relu, bias=bz_sb[:, mt:mt + 1])
                nc.sync.dma_start(zb_in[mt * 128:(mt + 1) * 128, :], zb[:])

            zb_out = dram.tile([H, B], bf16, tag="zbout", name=f"zbout_{t}",
                               addr_space="Shared")
            if ABLATE_CC:
                nc.sync.dma_start(zb_out[0:256, :], zb_in[:])
            else:
                nc.gpsimd.collective_compute(
                    "AllGather", mybir.AluOpType.bypass, replica_groups=RG,
                    ins=[zb_in.opt()], outs=[zb_out.opt()],
                )
            zcat = state.tile([128, KT * B], bf16, tag="zcat", name=f"zcat_{t}")
            load_cat(zcat, zb_out)

            # ---- fc2 (full 165 rows, replicated) + out store + bf16 copy
            new_outb = []
            for mt in range(2):
                mr = MR[mt]
                ps = psum.tile([128, B], f32, tag="ps", name=f"pso_{t}_{mt}")
                for ki in range(KT):
                    nc.tensor.matmul(
                        ps[:mr],
                        wfc2_sb[:, ki * OUT + mt * 128: ki * OUT + mt * 128 + mr],
                        zcat[:, ki * B:(ki + 1) * B],
                        start=(ki == 0),
                        stop=(ki == KT - 1),
                    )
                of = work.tile([128, B], f32, tag=f"of{mt}", name=f"of_{t}_{mt}")
                nc.scalar.activation(of[:mr], ps[:mr], AF.Identity, bias=bo_sb[:mr, mt:mt + 1])
                if not ABLATE_STORE:
                    nc.gpsimd.dma_start(
                        outs_d.ap()[t, mt * 128: mt * 128 + mr, :],
                        of[:mr, ds(col0, BL)],
                    )
                ob = state.tile([mr, B], bf16, tag=f"outb{mt}", name=f"outb{mt}_{t}")
                nc.vector.tensor_copy(ob[:], of[:mr])
                new_outb.append(ob)
            outb0, outb1 = new_outb
            hcat = hcat_new

    nc.compile()
    return nc


def _prepare_in_maps(inputs):
    bf = ml_dtypes.bfloat16
    f = {k: np.asarray(v) for k, v in inputs.items()}
    W_enc = f["W_enc"].astype(np.float32)
    b_enc = f["b_enc"].astype(np.float32)
    W_ih = f["W_ih"].astype(np.float32)
    b_ih = f["b_ih"].astype(np.float32)
    W_hh = f["W_hh"].astype(np.float32)
    b_hh = f["b_hh"].astype(np.float32)
    W_fc1 = f["W_fc1"].astype(np.float32)
    b_fc1 = f["b_fc1"].astype(np.float32)
    W_fc2 = f["W_fc2"].astype(np.float32)
    b_fc2 = f["b_fc2"].astype(np.float32)
    W_inh = f["W_inh"].astype(np.float32)
    b_inh = f["b_inh"].astype(np.float32)
    W_inc = f["W_inc"].astype(np.float32)
    b_inc = f["b_inc"].astype(np.float32)
    labels = f["labels"].astype(np.int64)
    x = f["inputs"].astype(np.float32)

    frame0 = x.reshape(B, OUT)
    h0 = frame0 @ W_inh.T + b_inh            # [B, H]
    c0 = frame0 @ W_inc.T + b_inc            # [B, H]
    onehot = np.zeros((NCLS, B), np.float32)
    onehot[labels, np.arange(B)] = 1.0
    M1 = W_ih[:, OUT:] @ W_enc               # [4H, NCLS]
    bias_gates = b_ih + b_hh + W_ih[:, OUT:] @ b_enc  # [4H]

    in_maps = []
    for j in range(NC):
        mt = np.arange(8)
        gt, g = mt % 4, mt // 4
        rows = (gt[:, None] * H + j * 256 + g[:, None] * 128 + np.arange(128)[None, :]).reshape(-1)
        zrows = j * 256 + np.arange(256)
        bg = bias_gates[rows].reshape(8, 128).T.copy()          # [128, 8]
        bzv = b_fc1[zrows].reshape(2, 128).T.copy()             # [128, 2]
        bov = np.zeros((128, 2), np.float32)
        bov[:, 0] = b_fc2[:128]
        bov[:MR[1], 1] = b_fc2[128:]
        in_maps.append({
            "whh": np.ascontiguousarray(W_hh[rows].T).astype(bf),
            "wih": np.ascontiguousarray(W_ih[rows, :OUT].T).astype(bf),
            "moh": np.ascontiguousarray(M1[rows].T).astype(bf),
            "wfc1": np.ascontiguousarray(W_fc1[zrows].T).astype(bf),
            "wfc2": np.ascontiguousarray(W_fc2.T).astype(bf),
            "onehot": onehot.astype(bf),
            "bgates": bg,
            "bz": bzv,
            "bo": bov,
            "h0": np.ascontiguousarray(h0.T).astype(bf),
            "c0": np.ascontiguousarray(c0.T[zrows]).astype(np.float32),
            "out0": np.ascontiguousarray(frame0.T).astype(bf),
        })
    return in_maps


def _get_program(L):
    if L not in _CACHE:
        _CACHE[L] = _build(L)
    return _CACHE[L]


def kernel(**inputs):
    from concourse.bass_utils import run_bass_kernel_spmd

    L = int(np.asarray(inputs["length"]))
    x = np.asarray(inputs["inputs"])
    Bq, J, D = x.shape
    assert (Bq, J * D) == (B, OUT)

    nc = _get_program(L)
    in_maps = _prepare_in_maps(inputs)
    res = run_bass_kernel_spmd(nc, in_maps, core_ids=list(range(NC)))
    # core j returns [L, OUT, BL] covering batch columns j*BL:(j+1)*BL
    full = np.concatenate([res.results[j]["outs"] for j in range(NC)], axis=2)
    out = np.transpose(full, (2, 0, 1)).reshape(B, L, J, D).astype(np.float32)
    return out
